# revision 52
# baseline (speedup 1.0000x reference)
"""NT-Xent contrastive loss on 8 Trainium2 NeuronCores — symmetric fp8 version.

Math: z = l2-normalize rows of concat(emb_i, emb_j) -> [8192, 512].
sim = (z @ z.T)/T, T=0.5.  denom_r = sum_j exp(sim_rj) - exp(sim_rr).
loss = (sum_r log denom_r - 4*sum_k cos_k) / 8192.

exp(sim) is symmetric, so only the upper triangle of the 16x16 grid of
512-row strip pairs (136 pairs) is computed, 17 pairs/core: core c owns
row strips A=2c, B=2c+1 and multiplies them against 10 column strips
(A, B, the 6 cyclically-next strips c1..c6, and two "far" strips x,y
that split the distance-4 superblock between core pairs).  Per block
both reductions are needed: row sums into denom of the row strip (free
via the exp ACTIVATE's accum_out — its READ_ACCUMULATOR overlaps the
next ACTIVATE), and column sums into denom of the col strip (mirror):
DVE accumulates exp blocks elementwise into bf16 SBUF accumulators
which the host reduces over partitions.  Diagonal blocks appear only
in row sums (host subtracts the exact self term).

The scalar engine is the critical resource — it must exp all 4.45M
computed elements per core (~29us at 1 elem/lane/cycle @1.2GHz) — so
the program is one dense chain of 20 full-width exp ops; matmuls
(DoubleRow fp8, 2x PE rate, K=512 as 2 double-chunks of the
[128,4,cols] ksub layout) run ahead of it.  Head: dummy warm-up
matmuls flip the HAM clock gate during the input-DMA wait, the first
wave needs only the first 512KB chunk, and input chunks 1+2 use
parallel SDMA rings (parallel DMAs get more aggregate bandwidth).
Tail: the last wave touches only small accumulators.

Host does the O(N*D) work exactly in f64: normalization, fp8(e4m3)
quantization (x64 scale; TRN FP8_EXP4 == ml_dtypes float8_e4m3 for
|v|<240), positive pairs, per-row self-term, final log/assembly.
"""

import numpy as np
import ml_dtypes

import concourse.bacc as bacc
import concourse.bass as bass
import concourse.tile as tile
from concourse import mybir
from concourse.bass_utils import run_bass_kernel_spmd

F32 = mybir.dt.float32
BF16 = mybir.dt.bfloat16
F8 = mybir.dt.float8e4
AF = mybir.ActivationFunctionType
ALU = mybir.AluOpType
ts = bass.ts

N_CORES = 8
N = 4096
D = 512
M = 2 * N
SW = 512                 # strip width (rows)
KSUB = D // 128          # 4 k-subtiles of 128
SCALE = 64.0             # fp8 quantization scale for z
ACT_SCALE = 2.0 / (SCALE * SCALE)   # exp(sim_psum * ACT_SCALE) = exp(2*cos)
NCOL = 10 * SW

# local col-tile order in zt / SBUF: [T0 | T1 T2 T3 | T4 | T5 T6 T7 T8 T9]
TILE_OFF = {0: 0, 1: 512, 2: 1024, 3: 1536, 4: 2048, 5: 2560,
            6: 3072, 7: 3584, 8: 4096, 9: 4608}
CH1, CH2A, CH2B, CH3 = 512, 1536, 512, 2560     # DMA chunk widths


def strips_for_core(c):
    base = [(2 * c + i) % 16 for i in range(8)]
    if c < 4:
        x, y = 2 * c + 8, 2 * c + 9
    else:
        x, y = 2 * c - 7, 2 * c - 8
    return base + [x, y]


def build_program():
    nc = bacc.Bacc(
        "TRN2",
        target_bir_lowering=False,
        debug=False,
        num_devices=N_CORES,
    )

    zt1_d = nc.dram_tensor("zt1", [128, KSUB, 512], F8, kind="ExternalInput")
    zt2a_d = nc.dram_tensor("zt2a", [128, KSUB, CH2A], F8,
                            kind="ExternalInput")
    zt2b_d = nc.dram_tensor("zt2b", [128, KSUB, CH2B], F8,
                            kind="ExternalInput")
    zt3_d = nc.dram_tensor("zt3", [128, KSUB, 2560], F8,
                           kind="ExternalInput")
    rs_d = nc.dram_tensor("rs", [128, 8], F32, kind="ExternalOutput")
    accT1_d = nc.dram_tensor("accT1", [128, 512], BF16, kind="ExternalOutput")
    accC_d = nc.dram_tensor("accC", [128, 1536], BF16, kind="ExternalOutput")
    accDa_d = nc.dram_tensor("accDa", [128, 1536], BF16,
                             kind="ExternalOutput")
    accDb_d = nc.dram_tensor("accDb", [128, 512], BF16, kind="ExternalOutput")
    accT8_d = nc.dram_tensor("accT8", [128, 512], BF16, kind="ExternalOutput")

    DR = mybir.MatmulPerfMode.DoubleRow

    with tile.TileContext(nc) as tc:
        import contextlib

        with contextlib.ExitStack() as ctx:
            big = ctx.enter_context(tc.tile_pool(name="big", bufs=1))
            esp = ctx.enter_context(tc.tile_pool(name="esp", bufs=3))
            pp = ctx.enter_context(
                tc.tile_pool(name="pp", bufs=2, space="PSUM")
            )

            zt1 = big.tile([128, KSUB, CH1], F8, tag="zt1")
            zt2a = big.tile([128, KSUB, CH2A], F8, tag="zt2a")
            zt2b = big.tile([128, KSUB, CH2B], F8, tag="zt2b")
            zt3 = big.tile([128, KSUB, CH3], F8, tag="zt3")
            accT1 = big.tile([128, 512], BF16, tag="accT1")
            accC = big.tile([128, 1536], BF16, tag="accC")
            accDa = big.tile([128, 1536], BF16, tag="accDa")
            accDb = big.tile([128, 512], BF16, tag="accDb")
            accT8 = big.tile([128, 512], BF16, tag="accT8")
            dacc = big.tile([128, 32], F32, tag="dacc")
            rs = big.tile([128, 8], F32, tag="rs")

            # zero accumulators on gpsimd (keeps DVE free); earliest-
            # needed first
            nc.gpsimd.memset(dacc[:], 0.0)
            nc.gpsimd.memset(accT1[:], 0.0)
            nc.gpsimd.memset(accC[:], 0.0)
            nc.gpsimd.memset(accDa[:], 0.0)
            nc.gpsimd.memset(accDb[:], 0.0)
            nc.gpsimd.memset(accT8[:], 0.0)

            # chunks 1+2a+2b stream on three parallel rings (concurrent
            # DMAs use separate rings -> more aggregate bandwidth); each
            # chunk is its own DRAM tensor so every transfer is fully
            # contiguous per partition (2-10KB segments, max ring bw);
            # chunk 3 has slack, so a corner-copy fake dep holds it off
            # until chunk 2b has landed
            nc.sync.dma_start(zt1[:], zt1_d[:])
            nc.sync.dma_start(zt2a[:], zt2a_d[:])
            nc.sync.dma_start(zt2b[:], zt2b_d[:])
            nc.vector.tensor_copy(zt3[0:1, 0:1, 0:1], zt2b[0:1, 0:1, 0:1])
            nc.sync.dma_start(zt3[:], zt3_d[:])

            # dummy matmuls during the input-DMA wait: ~4us of sustained
            # PE activity flips the HAM clock gate to 8/8 (2.4 GHz) so
            # the first real matmuls run warm instead of at 1.2 GHz.
            wsrc = big.tile([128, 512], BF16, tag="wsrc")
            nc.vector.memset(wsrc[:], 1.0)
            warm = pp.tile([128, 2048], F32, tag="pp", name="warm")
            for i in range(9):
                nc.tensor.matmul(
                    warm[:, ts(i % 4, 512)],
                    wsrc[:, 0:128], wsrc[:, 0:512],
                    start=True, stop=True,
                )

            def chunk_of(t):
                off = TILE_OFF[t]
                if off < CH1:
                    return zt1, off
                if off < CH1 + CH2A:
                    return zt2a, off - CH1
                if off < CH1 + CH2A + CH2B:
                    return zt2b, off - CH1 - CH2A
                return zt3, off - CH1 - CH2A - CH2B

            def emit_wave(wname, row, wslot, tiles_):
                """Rowgroups of 128 rows from strip row(0=A,1=B) x the
                col tiles in tiles_; psum slot i = tiles_[i].  The exp's
                accum_out writes rowsums to dacc slot (row*4+g)*4+wslot.
                Yields (g, es) after each rowgroup's exp."""
                nt = len(tiles_)
                w = nt * 512
                lcht, lbase = chunk_of(0 if row == 0 else 1)
                for g in range(4):
                    lhs_off = lbase + g * 128
                    pt = pp.tile([128, 2048], F32, tag="pp",
                                 name=f"pt_{wname}_{g}")
                    for k in range(2):
                        lhsT = lcht[:, 2 * k : 2 * k + 2,
                                    lhs_off : lhs_off + 128]
                        for i, t in enumerate(tiles_):
                            cht, choff = chunk_of(t)
                            rhs = cht[:, 2 * k : 2 * k + 2,
                                      choff : choff + 512]
                            nc.tensor.matmul(
                                pt[:, ts(i, 512)], lhsT, rhs,
                                start=(k == 0), stop=(k == 1),
                                perf_mode=DR,
                            )
                    es = esp.tile([128, 2048], BF16, tag="esp",
                                  name=f"es_{wname}_{g}")
                    gslot = (row * 4 + g) * 4 + wslot
                    nc.scalar.activation(
                        es[0:128, 0:w], pt[0:128, 0:w], AF.Exp,
                        scale=ACT_SCALE,
                        accum_out=dacc[:, gslot : gslot + 1],
                    )
                    yield g, es

            # ---- A1: rows A x [T0] (diag block: rowsums only, no
            #      mirror adds; needs only the 256KB first chunk) ----
            for g, es in emit_wave("a1", 0, 0, [0]):
                pass

            # ---- A2: rows A x [T1 T2 T3 T4]; mirrors all ----
            for g, es in emit_wave("a2", 0, 1, [1, 2, 3, 4]):
                nc.vector.tensor_add(accT1[:], accT1[:], es[:, 0:512])
                nc.vector.tensor_add(accC[:], accC[:], es[:, 512:2048])
            nc.sync.dma_start(accT1_d[:], accT1[:])

            # ---- B1: rows B x [T1 T2 T3 T4]; mirrors T2 T3 T4 ----
            for g, es in emit_wave("b1", 1, 0, [1, 2, 3, 4]):
                nc.vector.tensor_add(accC[:], accC[:], es[:, 512:2048])
            nc.sync.dma_start(accC_d[:], accC[:])

            # ---- A3: rows A x [T5 T6 T7 T8]; mirrors all ----
            for g, es in emit_wave("a3", 0, 2, [5, 6, 7, 8]):
                nc.vector.tensor_add(accDa[:], accDa[:], es[:, 0:1536])
                nc.vector.tensor_add(accT8[:], accT8[:], es[:, 1536:2048])
            nc.sync.dma_start(accT8_d[:], accT8[:])

            # ---- B2: rows B x [T5 T6 T7 T9]; mirrors all; last wave
            #      ships the small accs on two trigger queues ----
            for g, es in emit_wave("b2", 1, 1, [5, 6, 7, 9]):
                nc.vector.tensor_add(accDa[:], accDa[:], es[:, 0:1536])
                nc.vector.tensor_add(accDb[:], accDb[:], es[:, 1536:2048])
            nc.sync.dma_start(accDa_d[:], accDa[:])
            nc.scalar.dma_start(accDb_d[:], accDb[:])

            # ---- rowsum finale ----
            nc.vector.tensor_reduce(
                rs[:], dacc[:].rearrange("p (g w) -> p g w", w=4),
                axis=mybir.AxisListType.X, op=ALU.add,
            )
            nc.sync.dma_start(rs_d[:], rs[:])

    nc.compile()
    return nc


_NC_CACHE = None


def _get_program():
    global _NC_CACHE
    if _NC_CACHE is None:
        _NC_CACHE = build_program()
    return _NC_CACHE


def quantize_z(emb_i: np.ndarray, emb_j: np.ndarray):
    """Host-side exact prep: returns (q8 [8192,512] fp8, pos_sum, selfterm)."""
    reps = np.concatenate(
        [np.asarray(emb_i, np.float64), np.asarray(emb_j, np.float64)], 0
    )
    z = reps / np.linalg.norm(reps, axis=1, keepdims=True)
    q8 = (z * SCALE).astype(np.float32).astype(ml_dtypes.float8_e4m3)
    qf = q8.astype(np.float64) / SCALE
    pos_sum = float((z[:N] * z[N:]).sum())
    selfterm = np.exp(2.0 * (qf * qf).sum(1))        # device's own diag entry
    return q8, pos_sum, selfterm


def make_in_maps(q8: np.ndarray):
    # zt[p, ksub, col] = q8[global_col_row, ksub*128 + p]; one DRAM
    # tensor per DMA chunk so transfers are contiguous per partition
    qT = np.ascontiguousarray(q8.T).reshape(KSUB, 128, M)  # [ksub, p, row]
    in_maps = []
    order_idx = sorted(TILE_OFF, key=TILE_OFF.get)   # tile ids by offset
    bounds = [(0, CH1), (CH1, CH1 + CH2A),
              (CH1 + CH2A, CH1 + CH2A + CH2B),
              (CH1 + CH2A + CH2B, NCOL)]
    names = ["zt1", "zt2a", "zt2b", "zt3"]
    for c in range(N_CORES):
        S = strips_for_core(c)
        cols = np.concatenate(
            [np.arange(S[t] * SW, (S[t] + 1) * SW) for t in order_idx]
        )
        zt = np.ascontiguousarray(
            qT[:, :, cols].transpose(1, 0, 2)
        )  # [128, KSUB, NCOL]
        in_maps.append(
            {
                nm: np.ascontiguousarray(zt[:, :, lo:hi])
                for nm, (lo, hi) in zip(names, bounds)
            }
        )
    return in_maps


def combine_outputs(results, pos_sum, selfterm):
    denom = np.zeros(M, np.float64)
    for c in range(N_CORES):
        S = strips_for_core(c)
        A, B = S[0], S[1]
        r = results[c]
        rs = np.asarray(r["rs"], np.float64)             # [128, 8]
        denom[A * SW : (A + 1) * SW] += rs[:, 0:4].T.reshape(SW)
        denom[B * SW : (B + 1) * SW] += rs[:, 4:8].T.reshape(SW)
        csT1 = np.asarray(r["accT1"], np.float64).sum(0)   # [T1]
        csC = np.asarray(r["accC"], np.float64).sum(0)     # [T2 T3 T4]
        csDa = np.asarray(r["accDa"], np.float64).sum(0)   # [T5 T6 T7]
        csDb = np.asarray(r["accDb"], np.float64).sum(0)   # [T9]
        csT8 = np.asarray(r["accT8"], np.float64).sum(0)   # [T8]
        denom[S[1] * SW : (S[1] + 1) * SW] += csT1
        for i, t in enumerate([2, 3, 4]):
            g = S[t]
            denom[g * SW : (g + 1) * SW] += csC[i * 512 : (i + 1) * 512]
        for i, t in enumerate([5, 6, 7]):
            g = S[t]
            denom[g * SW : (g + 1) * SW] += csDa[i * 512 : (i + 1) * 512]
        denom[S[9] * SW : (S[9] + 1) * SW] += csDb
        denom[S[8] * SW : (S[8] + 1) * SW] += csT8
    denom -= selfterm
    loss = (np.log(denom).sum() - 4.0 * pos_sum) / float(M)
    return np.float32(loss)


def kernel(emb_i: np.ndarray, emb_j: np.ndarray) -> np.ndarray:
    nc = _get_program()
    q8, pos_sum, selfterm = quantize_z(emb_i, emb_j)
    in_maps = make_in_maps(q8)
    res = run_bass_kernel_spmd(nc, in_maps, list(range(N_CORES)))
    return combine_outputs(res.results, pos_sum, selfterm)
